# revision 8
# baseline (speedup 1.0000x reference)
"""MiniAttention Trainium2 Bass kernel.

Problem: B=8, N=1024, C=768, H=12, D=64.
  qkv = x @ w_qkv.T ; q,k,v heads ; S = (q*SCALE) @ k.T per head
  A1 = conv_l-mix over heads ; P = softmax_m(A1) ; A2 = conv_w-mix over heads
  out = (A2 @ v per head) @ w_proj.T + b_proj
Sharding: pure batch-parallel, 1 batch element per NeuronCore (8 cores).

Per-core design (PE matmuls in bf16, f32 accumulation):
  - Host passes x^T, w_qkv^T, w_proj^T (transposed on host, bf16).
  - QKV as two matmul orientations: q,k in [cout, n] layout, v in [n, cout].
  - Scores per head h, query-block nb: S_h [nb, 1024] in PSUM -> evac bf16
    into S_all [block_row, (h, m)] (all heads side by side on the free dim).
  - Head-interleave into groups of G=10 queries with row map r = rr*12 + h:
    ONE SBUF->SBUF DMA per group (dest sint[:, g, :], src
    S_all[g*G:(g+1)*G, :, :]) instead of one DMA per (head, group) --
    HWDGE dma_start costs ~600ns of engine occupancy each, so instruction
    count dominates; 13/block instead of 144/block.
  - conv_l (with SCALE folded) as constant rr-block-diagonal lhsT; exp on
    ACT with accum_out giving softmax sums for free; softmax normalization
    folded into the per-group conv_w lhsT (rows scaled by 1/sum).
  - A2 [rows, m] -> DMA-transpose (HWDGE xbar, bf16) into [m, rows] chunks
    so attention@V contracts m on the PE at K=128.
  - proj consumes the accumulated attn^T [768, 1024]; host re-transposes
    the [768, 1024] per-core output and adds b_proj.
"""

import numpy as np
import ml_dtypes

B, N, C, H = 8, 1024, 768, 12
D = C // H
SCALE = D ** -0.5
G = 10          # queries per mix group
NB = 120        # queries per block (12 groups)
NBLK = 8        # full blocks; last block is ragged: 6 groups of 10 + 1 of 4
BF16 = ml_dtypes.bfloat16

_cached = None


def _block_layout():
    """Returns list of blocks: (n0, nb, chunks) where chunks is a list of
    (row_start, g_start, g_count, g_size) describing the query groups."""
    blocks = []
    for b in range(NBLK):
        blocks.append((b * NB, NB, [(0, 0, 12, G)]))
    # ragged tail: n in [960, 1024) = 6 groups of 10 + 1 group of 4
    blocks.append((960, 64, [(0, 0, 6, G), (60, 6, 1, 4)]))
    return blocks


def _build_program():
    import concourse.tile as tile
    from concourse import bacc, mybir

    f32 = mybir.dt.float32
    bf16 = mybir.dt.bfloat16
    Exp = mybir.ActivationFunctionType.Exp

    nc = bacc.Bacc("TRN2", target_bir_lowering=False, debug=False)

    xt = nc.dram_tensor("xt", [C, N], bf16, kind="ExternalInput").ap()
    wqkvt = nc.dram_tensor("wqkvt", [C, 3 * C], bf16, kind="ExternalInput").ap()
    wprojt = nc.dram_tensor("wprojt", [C, C], bf16, kind="ExternalInput").ap()
    m1w_in = nc.dram_tensor("m1w", [12 * G, 12 * G], bf16, kind="ExternalInput").ap()
    m2p_in = nc.dram_tensor("m2p", [12 * G, 128], f32, kind="ExternalInput").ap()
    out_d = nc.dram_tensor("out", [C, N], f32, kind="ExternalOutput").ap()

    KC = C // 128  # 6 contraction chunks

    # evacuation engine round robin: ACT carries the exps, so it only gets
    # ~1/5 of the PSUM->SBUF copies; DVE takes the rest.
    _ec = [0]

    def evac(dst, src):
        i = _ec[0]
        _ec[0] += 1
        if i % 5 == 0:
            nc.scalar.copy(dst, src)
        else:
            nc.vector.tensor_copy(dst, src)

    with tile.TileContext(nc) as tc:
        with tc.tile_pool(name="const", bufs=1) as const, \
             tc.tile_pool(name="big", bufs=1) as big:

            m1wsb = const.tile([120, 120], bf16)
            nc.sync.dma_start(m1wsb, m1w_in)
            m2psb = const.tile([120, 128], f32)
            nc.sync.dma_start(m2psb, m2p_in)

            # persistent activations
            qksb = big.tile([128, 2 * KC, N], bf16)   # ct 0..5 = q, 6..11 = k
            vsb = big.tile([128, 8, C], bf16)         # [m%128, m//128, cout]
            attnT = big.tile([128, KC, N], bf16)      # [cout2%128, cout2//128, n]

            # ---------------- QKV ----------------
            with tc.tile_pool(name="xtp", bufs=1) as xtp, \
                 tc.tile_pool(name="qkvps", bufs=3, space="PSUM") as qkvps, \
                 tc.tile_pool(name="vps", bufs=2, space="PSUM") as vps:
                xtsb = xtp.tile([128, KC, N], bf16)
                nc.sync.dma_start(xtsb, xt.rearrange("(kc p) n -> p kc n", p=128))
                wqsb = xtp.tile([128, KC, 3 * C], bf16)
                nc.sync.dma_start(
                    wqsb, wqkvt.rearrange("(kc p) c -> p kc c", p=128))

                # q, k: out[cout_tile, n]
                for ct in range(12):
                    for nh in range(2):
                        ps = qkvps.tile([128, 512], f32, tag="qkv")
                        for kc in range(KC):
                            nc.tensor.matmul(
                                ps,
                                lhsT=wqsb[:, kc, 128 * ct:128 * ct + 128],
                                rhs=xtsb[:, kc, 512 * nh:512 * nh + 512],
                                start=(kc == 0), stop=(kc == KC - 1),
                            )
                        evac(qksb[:, ct, 512 * nh:512 * nh + 512], ps)

                # v: out[n_tile, cout]
                for nt in range(8):
                    ps = vps.tile([128, 768], f32, tag="vps")
                    for half, (c0, c1) in enumerate([(0, 512), (512, 768)]):
                        for kc in range(KC):
                            nc.tensor.matmul(
                                ps[:, c0:c1],
                                lhsT=xtsb[:, kc, 128 * nt:128 * nt + 128],
                                rhs=wqsb[:, kc, 2 * C + c0:2 * C + c1],
                                start=(kc == 0), stop=(kc == KC - 1),
                            )
                    evac(vsb[:, nt, :], ps)

            # ---------------- attention blocks ----------------
            with tc.tile_pool(name="sallp", bufs=1) as sallp, \
                 tc.tile_pool(name="sint", bufs=2) as sintp, \
                 tc.tile_pool(name="pint", bufs=5) as pintp, \
                 tc.tile_pool(name="a2p", bufs=5) as a2p, \
                 tc.tile_pool(name="a2tp", bufs=2) as a2tp, \
                 tc.tile_pool(name="smp", bufs=10) as smp, \
                 tc.tile_pool(name="m2wp", bufs=8) as m2wp, \
                 tc.tile_pool(name="psS", bufs=2, space="PSUM") as psS, \
                 tc.tile_pool(name="psA1", bufs=2, space="PSUM") as psA1, \
                 tc.tile_pool(name="psA2", bufs=2, space="PSUM") as psA2, \
                 tc.tile_pool(name="psAV", bufs=2, space="PSUM") as psAV:

                for (n0, nb, chunks) in _block_layout():
                    sall = sallp.tile([128, 12, N], bf16, tag="sall")

                    # --- scores per head, evac into S_all [row, (h, m)] ---
                    for h in range(12):
                        base = 64 * (h % 2)
                        for mh in range(2):
                            ps = psS.tile([128, 512], f32, tag="psS")
                            nc.tensor.matmul(
                                ps[0:nb, :],
                                lhsT=qksb[base:base + 64, h // 2, n0:n0 + nb],
                                rhs=qksb[base:base + 64, 6 + h // 2,
                                         512 * mh:512 * mh + 512],
                                start=True, stop=True,
                            )
                            evac(sall[0:nb, h, 512 * mh:512 * mh + 512],
                                 ps[0:nb, :])

                    # --- interleave: one DMA per group ---
                    # sint[rr*12+h, g, m] = S_all[g*gs+rr, h, m]
                    sint = sintp.tile([128, 12, N], bf16, tag="sint")
                    for (rs, g0, gc, gs) in chunks:
                        for gi in range(gc):
                            # SWDGE (gpsimd): keeps the HWDGE/Sync queue free
                            # for the xbar transposes; Pool engine is idle.
                            nc.gpsimd.dma_start(
                                out=sint[0:12 * gs, g0 + gi, :],
                                in_=sall[rs + gi * gs:rs + (gi + 1) * gs, :, :],
                            )

                    # --- per-group mix1 -> exp -> mix2 -> transpose ---
                    a2t = a2tp.tile([128, 12, 8, 128], bf16, tag="a2t")
                    for (rs, g0, gc, gs) in chunks:
                        rows = 12 * gs
                        for g in range(g0, g0 + gc):
                            pg = pintp.tile([128, N], bf16, tag="pint")
                            sm = smp.tile([128, 4], f32, tag="sm")
                            for mh in range(2):
                                a1 = psA1.tile([128, 512], f32, tag="psA1")
                                nc.tensor.matmul(
                                    a1[0:rows, :],
                                    lhsT=m1wsb[0:rows, 0:rows],
                                    rhs=sint[0:rows, g, 512 * mh:512 * mh + 512],
                                    start=True, stop=True,
                                )
                                nc.scalar.activation(
                                    pg[0:rows, 512 * mh:512 * mh + 512],
                                    a1[0:rows, :], Exp,
                                    accum_out=sm[0:rows, mh:mh + 1],
                                )
                            nc.gpsimd.tensor_add(
                                sm[0:rows, 2:3], sm[0:rows, 0:1], sm[0:rows, 1:2])
                            nc.vector.reciprocal(
                                sm[0:rows, 3:4], sm[0:rows, 2:3])
                            w2 = m2wp.tile([128, 128], bf16, tag="m2w")
                            nc.gpsimd.tensor_scalar_mul(
                                w2[0:rows, :], m2psb[0:rows, :], sm[0:rows, 3:4])
                            a2sb = a2p.tile([128, N], bf16, tag="a2sb")
                            for mh in range(2):
                                a2 = psA2.tile([128, 512], f32, tag="psA2")
                                nc.tensor.matmul(
                                    a2,
                                    lhsT=w2[0:rows, :],
                                    rhs=pg[0:rows, 512 * mh:512 * mh + 512],
                                    start=True, stop=True,
                                )
                                evac(a2sb[:, 512 * mh:512 * mh + 512], a2)
                            nc.sync.dma_start(
                                out=a2t[:, g, :, :], in_=a2sb,
                                transpose=True,
                            )

                    # --- attention @ V ---
                    # A2 row (rr, o) = rr*12 + o; head-o rows = o::12.
                    # Column-packed head pairs: head 2j -> av2[0:64] (col
                    # tile 0), head 2j+1 -> av2[64:128] (col tile 64); the
                    # two K=128 chains run concurrently on the PE array.
                    for j in range(6):
                        av2 = psAV.tile([128, 128], f32, tag="psAV")
                        for half in range(2):
                            o = 2 * j + half
                            for (rs, g0, gc, gs) in chunks:
                                for c in range(8):
                                    nc.tensor.matmul(
                                        av2[64 * half:64 * half + 64,
                                            rs:rs + gc * gs],
                                        lhsT=vsb[:, c, 64 * o:64 * o + 64],
                                        rhs=a2t[:, g0:g0 + gc, c,
                                                o:o + 12 * (gs - 1) + 1:12],
                                        start=(c == 0), stop=(c == 7),
                                    )
                        evac(attnT[:, j, n0:n0 + nb], av2[:, 0:nb])

            # ---------------- proj ----------------
            with tc.tile_pool(name="projps", bufs=3, space="PSUM") as pjp, \
                 tc.tile_pool(name="outp", bufs=3) as outp:
                wpsb = outp.tile([128, KC, C], bf16)
                nc.sync.dma_start(
                    wpsb, wprojt.rearrange("(kc p) c -> p kc c", p=128))
                od = out_d.rearrange("(ct p) n -> p ct n", p=128)
                for ct in range(KC):
                    for nh in range(2):
                        ps = pjp.tile([128, 512], f32, tag="pj")
                        for kc in range(KC):
                            nc.tensor.matmul(
                                ps,
                                lhsT=wpsb[:, kc, 128 * ct:128 * ct + 128],
                                rhs=attnT[:, kc, 512 * nh:512 * nh + 512],
                                start=(kc == 0), stop=(kc == KC - 1),
                            )
                        ob = outp.tile([128, 512], f32, tag="ob")
                        evac(ob, ps)
                        nc.sync.dma_start(
                            od[:, ct, 512 * nh:512 * nh + 512], ob)

    nc.compile()
    return nc


def _mix_weights(conv_l_w, conv_w_w):
    """Host-built mix lhsT matrices, row map r = rr*12 + h.

    m1w[rr*12+h, rr*12+o] = SCALE * conv_l[o, h]   (lhsT for mix1)
    m2p[rr*12+h, rr*12+o] = conv_w[o, h]           (pattern, f32; scaled
        per-group on device by 1/softmax_sum per row; 128 cols, cols >= 120
        are zero so A2 psum rows [rows:128) are zeros)
    The gs=4 ragged group uses the leading [48, 48] / [48, :] slices.
    """
    m1 = np.zeros((120, 120), np.float32)
    m2 = np.zeros((120, 128), np.float32)
    for rr in range(G):
        for h in range(12):
            for o in range(12):
                m1[rr * 12 + h, rr * 12 + o] = SCALE * conv_l_w[o, h]
                m2[rr * 12 + h, rr * 12 + o] = conv_w_w[o, h]
    return m1.astype(BF16), m2.astype(np.float32)


def _run(x, w_qkv, w_proj, b_proj, conv_l_w, conv_w_w, **spmd_kwargs):
    global _cached
    from concourse import bass_utils

    x = np.asarray(x, np.float32)
    w_qkv = np.asarray(w_qkv, np.float32)
    w_proj = np.asarray(w_proj, np.float32)
    b_proj = np.asarray(b_proj, np.float32)
    conv_l_w = np.asarray(conv_l_w, np.float32)
    conv_w_w = np.asarray(conv_w_w, np.float32)

    if _cached is None:
        _cached = _build_program()
    nc = _cached

    m1w, m2p = _mix_weights(conv_l_w, conv_w_w)
    wqkvt = np.ascontiguousarray(w_qkv.T).astype(BF16)
    wprojt = np.ascontiguousarray(w_proj.T).astype(BF16)

    in_maps = []
    for b in range(B):
        in_maps.append({
            "xt": np.ascontiguousarray(x[b].T).astype(BF16),
            "wqkvt": wqkvt,
            "wprojt": wprojt,
            "m1w": m1w,
            "m2p": m2p,
        })

    res = bass_utils.run_bass_kernel_spmd(
        nc, in_maps, core_ids=list(range(B)), **spmd_kwargs)
    out = np.stack([res.results[b]["out"].T for b in range(B)])  # [B, N, C]
    return (out + b_proj[None, None, :]).astype(np.float32), res


def kernel(x, w_qkv, w_proj, b_proj, conv_l_w, conv_w_w):
    out, _ = _run(x, w_qkv, w_proj, b_proj, conv_l_w, conv_w_w)
    return out


# revision 10
# speedup vs baseline: 1.1236x; 1.1236x over previous
"""MiniAttention Trainium2 Bass kernel.

Problem: B=8, N=1024, C=768, H=12, D=64.
  qkv = x @ w_qkv.T ; q,k,v heads ; S = (q*SCALE) @ k.T per head
  A1 = conv_l-mix over heads ; P = softmax_m(A1) ; A2 = conv_w-mix over heads
  out = (A2 @ v per head) @ w_proj.T + b_proj
Sharding: pure batch-parallel, 1 batch element per NeuronCore (8 cores).

Per-core design (PE matmuls in bf16, f32 accumulation):
  - Host passes x^T, w_qkv^T, w_proj^T (transposed on host, bf16).
  - Scores per head h evac'd into S_all [block_row, (h, m)]; head-interleave
    into groups of G=10 queries with row map r = rr*12 + h via ONE SWDGE
    DMA per group (engines execute in order, and HWDGE dma_start costs
    ~600ns of Sync occupancy each, so both queue choice and instruction
    count matter).
  - conv_l (SCALE folded) as constant rr-block-diagonal lhsT; exp on ACT
    with accum_out giving softmax sums; normalization folded into the
    per-group conv_w lhsT (rows scaled by 1/sum, built on GpSimd).
  - A2 -> xbar DMA-transpose -> attention@V contracts m at K=128 with
    column-packed head pairs (full 128-partition PSUM tile).
  - All engines execute their instruction streams IN ORDER, so the block
    loop is software-pipelined by emission order: per iteration emit
    mixes(b-1) with a 3-half skew (mix2 lags mix1 so ACT exp latency is
    hidden) and the AV chains of block b-2 injected between mix steps,
    then scores(b) last. PSUM: scores+mix1 share one 4-buffer ring
    (tag "ps"), mix2 2 banks, AV 2 banks = 8 banks total.
  - PSUM->SBUF evacs are greedily balanced between ACT and DVE by
    accumulated estimated cost (exp is pinned to ACT, small ops to GpSimd).
"""

import numpy as np
import ml_dtypes

B, N, C, H = 8, 1024, 768, 12
D = C // H
SCALE = D ** -0.5
G = 10          # queries per mix group
NB = 120        # queries per block (12 groups)
NBLK = 8        # full blocks; last block is ragged: 6 groups of 10 + 1 of 4
BF16 = ml_dtypes.bfloat16

_cached = None


def _block_layout():
    """Returns list of blocks: (n0, nb, chunks) where chunks is a list of
    (row_start, g_start, g_count, g_size) describing the query groups."""
    blocks = []
    for b in range(NBLK):
        blocks.append((b * NB, NB, [(0, 0, 12, G)]))
    # ragged tail: n in [960, 1024) = 6 groups of 10 + 1 group of 4
    blocks.append((960, 64, [(0, 0, 6, G), (60, 6, 1, 4)]))
    return blocks


def _build_program():
    import concourse.tile as tile
    from concourse import bacc, mybir

    f32 = mybir.dt.float32
    bf16 = mybir.dt.bfloat16
    Exp = mybir.ActivationFunctionType.Exp

    nc = bacc.Bacc("TRN2", target_bir_lowering=False, debug=False)

    xt = nc.dram_tensor("xt", [C, N], bf16, kind="ExternalInput").ap()
    wqkvt = nc.dram_tensor("wqkvt", [C, 3 * C], bf16, kind="ExternalInput").ap()
    wprojt = nc.dram_tensor("wprojt", [C, C], bf16, kind="ExternalInput").ap()
    m1w_in = nc.dram_tensor("m1w", [12 * G, 12 * G], bf16, kind="ExternalInput").ap()
    m2p_in = nc.dram_tensor("m2p", [12 * G, 128], f32, kind="ExternalInput").ap()
    out_d = nc.dram_tensor("out", [C, N], f32, kind="ExternalOutput").ap()

    KC = C // 128  # 6 contraction chunks

    # greedy ACT/DVE balance by estimated occupancy (ns)
    _load = {"act": 0.0, "dve": 0.0}

    def evac(dst, src, n):
        """PSUM->SBUF copy of [rows, n]; pick the less-loaded engine."""
        if _load["act"] + 200 + n / 1.2 <= _load["dve"] + 200 + n / 0.96:
            _load["act"] += 200 + n / 1.2
            nc.scalar.copy(dst, src)
        else:
            _load["dve"] += 200 + n / 0.96
            nc.vector.tensor_copy(dst, src)

    with tile.TileContext(nc) as tc:
        with tc.tile_pool(name="const", bufs=1) as const, \
             tc.tile_pool(name="big", bufs=1) as big:

            m1wsb = const.tile([120, 120], bf16)
            nc.sync.dma_start(m1wsb, m1w_in)
            m2psb = const.tile([120, 128], f32)
            nc.sync.dma_start(m2psb, m2p_in)

            # persistent activations
            qksb = big.tile([128, 2 * KC, N], bf16)   # ct 0..5 = q, 6..11 = k
            vsb = big.tile([128, 8, C], bf16)         # [m%128, m//128, cout]
            attnT = big.tile([128, KC, N], bf16)      # [cout2%128, cout2//128, n]

            # ---------------- QKV ----------------
            with tc.tile_pool(name="xtp", bufs=1) as xtp, \
                 tc.tile_pool(name="qkvps", bufs=3, space="PSUM") as qkvps, \
                 tc.tile_pool(name="vps", bufs=2, space="PSUM") as vps:
                xtsb = xtp.tile([128, KC, N], bf16)
                nc.sync.dma_start(xtsb, xt.rearrange("(kc p) n -> p kc n", p=128))
                wqsb = xtp.tile([128, KC, 3 * C], bf16)
                nc.sync.dma_start(
                    wqsb, wqkvt.rearrange("(kc p) c -> p kc c", p=128))

                # q, k: out[cout_tile, n]
                for ct in range(12):
                    for nh in range(2):
                        ps = qkvps.tile([128, 512], f32, tag="qkv")
                        for kc in range(KC):
                            nc.tensor.matmul(
                                ps,
                                lhsT=wqsb[:, kc, 128 * ct:128 * ct + 128],
                                rhs=xtsb[:, kc, 512 * nh:512 * nh + 512],
                                start=(kc == 0), stop=(kc == KC - 1),
                            )
                        evac(qksb[:, ct, 512 * nh:512 * nh + 512], ps, 512)

                # v: out[n_tile, cout]
                for nt in range(8):
                    ps = vps.tile([128, 768], f32, tag="vps")
                    for half, (c0, c1) in enumerate([(0, 512), (512, 768)]):
                        for kc in range(KC):
                            nc.tensor.matmul(
                                ps[:, c0:c1],
                                lhsT=xtsb[:, kc, 128 * nt:128 * nt + 128],
                                rhs=wqsb[:, kc, 2 * C + c0:2 * C + c1],
                                start=(kc == 0), stop=(kc == KC - 1),
                            )
                    evac(vsb[:, nt, :], ps, 768)

            # ------- attention: software-pipelined over blocks -------
            blocks = _block_layout()
            nblk = len(blocks)

            with tc.tile_pool(name="sallp", bufs=1) as sallp, \
                 tc.tile_pool(name="sintp", bufs=2) as sintp, \
                 tc.tile_pool(name="pintp", bufs=4) as pintp, \
                 tc.tile_pool(name="a2p", bufs=4) as a2p, \
                 tc.tile_pool(name="a2tp", bufs=2) as a2tp, \
                 tc.tile_pool(name="smp", bufs=8) as smp, \
                 tc.tile_pool(name="m2wp", bufs=6) as m2wp, \
                 tc.tile_pool(name="psmix", bufs=4, space="PSUM") as psmix, \
                 tc.tile_pool(name="psA2", bufs=2, space="PSUM") as psA2, \
                 tc.tile_pool(name="psAV", bufs=2, space="PSUM") as psAV:

                sint_t = {}
                a2t_t = {}

                def stage_S(bi):
                    """Scores of block bi + evac into S_all + interleave."""
                    n0, nb, chunks = blocks[bi]
                    sall = sallp.tile([128, 12, N], bf16, tag="sall")
                    for h in range(12):
                        base = 64 * (h % 2)
                        for mh in range(2):
                            ps = psmix.tile([128, 512], f32, tag="ps")
                            nc.tensor.matmul(
                                ps[0:nb, :],
                                lhsT=qksb[base:base + 64, h // 2, n0:n0 + nb],
                                rhs=qksb[base:base + 64, 6 + h // 2,
                                         512 * mh:512 * mh + 512],
                                start=True, stop=True,
                            )
                            evac(sall[0:nb, h, 512 * mh:512 * mh + 512],
                                 ps[0:nb, :], 512)
                    # interleave: sint[rr*12+h, g, m] = S_all[g*gs+rr, h, m]
                    sint = sintp.tile([128, 12, N], bf16, tag="sint")
                    sint_t[bi] = sint
                    for (rs, g0, gc, gs) in chunks:
                        for gi in range(gc):
                            nc.gpsimd.dma_start(
                                out=sint[0:12 * gs, g0 + gi, :],
                                in_=sall[rs + gi * gs:rs + (gi + 1) * gs, :, :],
                            )

                def stage_M_gen(bi):
                    """Mixes of block bi, pipelined at (group, half) steps:
                    mix2 lags mix1 by SKEW steps to hide ACT exp latency."""
                    n0, nb, chunks = blocks[bi]
                    sint = sint_t.pop(bi)
                    a2t = a2tp.tile([128, 12, 8, 128], bf16, tag="a2t")
                    a2t_t[bi] = a2t
                    steps = []
                    for (rs, g0, gc, gs) in chunks:
                        for g in range(g0, g0 + gc):
                            for mh in range(2):
                                steps.append((g, gs, mh))
                    st = {}

                    def mix1_half(g, gs, mh):
                        rows = 12 * gs
                        if g not in st:
                            st[g] = {
                                "pg": pintp.tile([128, N], bf16, tag="pint", name="pg"),
                                "sm": smp.tile([128, 4], f32, tag="sm", name="sm"),
                                "w2": None, "a2sb": None,
                            }
                        s = st[g]
                        a1 = psmix.tile([128, 512], f32, tag="ps")
                        nc.tensor.matmul(
                            a1[0:rows, :],
                            lhsT=m1wsb[0:rows, 0:rows],
                            rhs=sint[0:rows, g, 512 * mh:512 * mh + 512],
                            start=True, stop=True,
                        )
                        _load["act"] += 620
                        nc.scalar.activation(
                            s["pg"][0:rows, 512 * mh:512 * mh + 512],
                            a1[0:rows, :], Exp,
                            accum_out=s["sm"][0:rows, mh:mh + 1],
                        )
                        if mh == 1:
                            sm = s["sm"]
                            nc.gpsimd.tensor_add(
                                sm[0:rows, 2:3], sm[0:rows, 0:1],
                                sm[0:rows, 1:2])
                            _load["dve"] += 150
                            nc.vector.reciprocal(
                                sm[0:rows, 3:4], sm[0:rows, 2:3])
                            w2 = m2wp.tile([128, 128], bf16, tag="m2w")
                            s["w2"] = w2
                            nc.gpsimd.tensor_scalar_mul(
                                w2[0:rows, :], m2psb[0:rows, :],
                                sm[0:rows, 3:4])

                    def mix2_half(g, gs, mh):
                        rows = 12 * gs
                        s = st[g]
                        if s["a2sb"] is None:
                            s["a2sb"] = a2p.tile([128, N], bf16, tag="a2sb", name="a2sb")
                        a2 = psA2.tile([128, 512], f32, tag="psA2")
                        nc.tensor.matmul(
                            a2,
                            lhsT=s["w2"][0:rows, :],
                            rhs=s["pg"][0:rows, 512 * mh:512 * mh + 512],
                            start=True, stop=True,
                        )
                        evac(s["a2sb"][:, 512 * mh:512 * mh + 512], a2, 512)
                        if mh == 1:
                            nc.sync.dma_start(
                                out=a2t[:, g, :, :], in_=s["a2sb"],
                                transpose=True,
                            )
                            del st[g]

                    SKEW = 3
                    for i in range(len(steps) + SKEW):
                        if i < len(steps):
                            mix1_half(*steps[i])
                        if i >= SKEW:
                            mix2_half(*steps[i - SKEW])
                        yield

                def av_thunks(bi):
                    """attention@V of block bi as 6 head-pair chain thunks."""
                    n0, nb, chunks = blocks[bi]
                    a2t = a2t_t.pop(bi)
                    out = []

                    def mk(j):
                        def emit():
                            av2 = psAV.tile([128, 128], f32, tag="psAV")
                            for half in range(2):
                                o = 2 * j + half
                                for (rs, g0, gc, gs) in chunks:
                                    for c in range(8):
                                        nc.tensor.matmul(
                                            av2[64 * half:64 * half + 64,
                                                rs:rs + gc * gs],
                                            lhsT=vsb[:, c, 64 * o:64 * o + 64],
                                            rhs=a2t[:, g0:g0 + gc, c,
                                                    o:o + 12 * (gs - 1) + 1:12],
                                            start=(c == 0), stop=(c == 7),
                                        )
                            evac(attnT[:, j, n0:n0 + nb], av2[:, 0:nb], nb)
                        return emit

                    for j in range(6):
                        out.append(mk(j))
                    return out

                for it in range(nblk + 2):
                    gen = stage_M_gen(it - 1) if 1 <= it <= nblk else None
                    avs = av_thunks(it - 2) if 2 <= it <= nblk + 1 else []
                    if gen is None:
                        for t in avs:
                            t()
                    else:
                        k, ai = 0, 0
                        for _ in gen:
                            k += 1
                            if ai < len(avs) and k % 4 == 0:
                                avs[ai]()
                                ai += 1
                        while ai < len(avs):
                            avs[ai]()
                            ai += 1
                    if it < nblk:
                        stage_S(it)

            # ---------------- proj ----------------
            with tc.tile_pool(name="projps", bufs=3, space="PSUM") as pjp, \
                 tc.tile_pool(name="outp", bufs=3) as outp:
                wpsb = outp.tile([128, KC, C], bf16)
                nc.sync.dma_start(
                    wpsb, wprojt.rearrange("(kc p) c -> p kc c", p=128))
                od = out_d.rearrange("(ct p) n -> p ct n", p=128)
                for ct in range(KC):
                    for nh in range(2):
                        ps = pjp.tile([128, 512], f32, tag="pj")
                        for kc in range(KC):
                            nc.tensor.matmul(
                                ps,
                                lhsT=wpsb[:, kc, 128 * ct:128 * ct + 128],
                                rhs=attnT[:, kc, 512 * nh:512 * nh + 512],
                                start=(kc == 0), stop=(kc == KC - 1),
                            )
                        ob = outp.tile([128, 512], f32, tag="ob")
                        evac(ob, ps, 512)
                        nc.sync.dma_start(
                            od[:, ct, 512 * nh:512 * nh + 512], ob)

    nc.compile()
    return nc


def _mix_weights(conv_l_w, conv_w_w):
    """Host-built mix lhsT matrices, row map r = rr*12 + h.

    m1w[rr*12+h, rr*12+o] = SCALE * conv_l[o, h]   (lhsT for mix1)
    m2p[rr*12+h, rr*12+o] = conv_w[o, h]           (pattern, f32; scaled
        per-group on device by 1/softmax_sum per row; 128 cols, cols >= 120
        are zero so A2 psum rows [rows:128) are zeros)
    The gs=4 ragged group uses the leading [48, 48] / [48, :] slices.
    """
    m1 = np.zeros((120, 120), np.float32)
    m2 = np.zeros((120, 128), np.float32)
    for rr in range(G):
        for h in range(12):
            for o in range(12):
                m1[rr * 12 + h, rr * 12 + o] = SCALE * conv_l_w[o, h]
                m2[rr * 12 + h, rr * 12 + o] = conv_w_w[o, h]
    return m1.astype(BF16), m2.astype(np.float32)


def _run(x, w_qkv, w_proj, b_proj, conv_l_w, conv_w_w, **spmd_kwargs):
    global _cached
    from concourse import bass_utils

    x = np.asarray(x, np.float32)
    w_qkv = np.asarray(w_qkv, np.float32)
    w_proj = np.asarray(w_proj, np.float32)
    b_proj = np.asarray(b_proj, np.float32)
    conv_l_w = np.asarray(conv_l_w, np.float32)
    conv_w_w = np.asarray(conv_w_w, np.float32)

    if _cached is None:
        _cached = _build_program()
    nc = _cached

    m1w, m2p = _mix_weights(conv_l_w, conv_w_w)
    wqkvt = np.ascontiguousarray(w_qkv.T).astype(BF16)
    wprojt = np.ascontiguousarray(w_proj.T).astype(BF16)

    in_maps = []
    for b in range(B):
        in_maps.append({
            "xt": np.ascontiguousarray(x[b].T).astype(BF16),
            "wqkvt": wqkvt,
            "wprojt": wprojt,
            "m1w": m1w,
            "m2p": m2p,
        })

    res = bass_utils.run_bass_kernel_spmd(
        nc, in_maps, core_ids=list(range(B)), **spmd_kwargs)
    out = np.stack([res.results[b]["out"].T for b in range(B)])  # [B, N, C]
    return (out + b_proj[None, None, :]).astype(np.float32), res


def kernel(x, w_qkv, w_proj, b_proj, conv_l_w, conv_w_w):
    out, _ = _run(x, w_qkv, w_proj, b_proj, conv_l_w, conv_w_w)
    return out


# revision 11
# speedup vs baseline: 1.1599x; 1.0323x over previous
"""MiniAttention Trainium2 Bass kernel.

Problem: B=8, N=1024, C=768, H=12, D=64.
  qkv = x @ w_qkv.T ; q,k,v heads ; S = (q*SCALE) @ k.T per head
  A1 = conv_l-mix over heads ; P = softmax_m(A1) ; A2 = conv_w-mix over heads
  out = (A2 @ v per head) @ w_proj.T + b_proj
Sharding: pure batch-parallel, 1 batch element per NeuronCore (8 cores).

Per-core design (PE matmuls in bf16, f32 accumulation):
  - Host passes x^T, w_qkv^T, w_proj^T (transposed on host, bf16).
  - Scores per head h evac'd into S_all [block_row, (h, m)]; head-interleave
    into groups of G=10 queries with row map r = rr*12 + h via ONE SWDGE
    DMA per group (engines execute in order, and HWDGE dma_start costs
    ~600ns of Sync occupancy each, so both queue choice and instruction
    count matter).
  - conv_l (SCALE folded) as constant rr-block-diagonal lhsT; exp on ACT
    with accum_out giving softmax sums; normalization folded into the
    per-group conv_w lhsT (rows scaled by 1/sum, built on GpSimd).
  - A2 -> xbar DMA-transpose -> attention@V contracts m at K=128 with
    column-packed head pairs (full 128-partition PSUM tile).
  - All engines execute their instruction streams IN ORDER, so the block
    loop is software-pipelined by emission order: per iteration emit
    mixes(b-1) with a 3-half skew (mix2 lags mix1 so ACT exp latency is
    hidden) and the AV chains of block b-2 injected between mix steps,
    then scores(b) last. PSUM: scores+mix1 share one 4-buffer ring
    (tag "ps"), mix2 2 banks, AV 2 banks = 8 banks total.
  - PSUM->SBUF evacs are greedily balanced between ACT and DVE by
    accumulated estimated cost (exp is pinned to ACT, small ops to GpSimd).
"""

import numpy as np
import ml_dtypes

B, N, C, H = 8, 1024, 768, 12
D = C // H
SCALE = D ** -0.5
G = 10          # queries per mix group
NB = 120        # queries per block (12 groups)
NBLK = 8        # full blocks; last block is ragged: 6 groups of 10 + 1 of 4
BF16 = ml_dtypes.bfloat16

_cached = None


def _block_layout():
    """Returns list of blocks: (n0, nb, chunks) where chunks is a list of
    (row_start, g_start, g_count, g_size) describing the query groups."""
    blocks = []
    for b in range(NBLK):
        blocks.append((b * NB, NB, [(0, 0, 12, G)]))
    # ragged tail: n in [960, 1024) = 6 groups of 10 + 1 group of 4
    blocks.append((960, 64, [(0, 0, 6, G), (60, 6, 1, 4)]))
    return blocks


def _build_program():
    import concourse.tile as tile
    from concourse import bacc, mybir

    f32 = mybir.dt.float32
    bf16 = mybir.dt.bfloat16
    Exp = mybir.ActivationFunctionType.Exp

    nc = bacc.Bacc("TRN2", target_bir_lowering=False, debug=False)

    xt = nc.dram_tensor("xt", [C, N], bf16, kind="ExternalInput").ap()
    wqkvt = nc.dram_tensor("wqkvt", [C, 3 * C], bf16, kind="ExternalInput").ap()
    wprojt = nc.dram_tensor("wprojt", [C, C], bf16, kind="ExternalInput").ap()
    m1w_in = nc.dram_tensor("m1w", [12 * G, 12 * G], bf16, kind="ExternalInput").ap()
    m2p_in = nc.dram_tensor("m2p", [12 * G, 128], f32, kind="ExternalInput").ap()
    out_d = nc.dram_tensor("out", [C, N], f32, kind="ExternalOutput").ap()

    KC = C // 128  # 6 contraction chunks

    # greedy ACT/DVE balance by estimated occupancy (ns)
    _load = {"act": 0.0, "dve": 0.0}

    def evac(dst, src, n):
        """PSUM->SBUF copy of [rows, n]; pick the less-loaded engine."""
        if _load["act"] + 200 + n / 1.2 <= _load["dve"] + 200 + n / 0.96:
            _load["act"] += 200 + n / 1.2
            nc.scalar.copy(dst, src)
        else:
            _load["dve"] += 200 + n / 0.96
            nc.vector.tensor_copy(dst, src)

    with tile.TileContext(nc) as tc:
        with tc.tile_pool(name="const", bufs=1) as const, \
             tc.tile_pool(name="big", bufs=1) as big:

            m1wsb = const.tile([120, 120], bf16)
            nc.sync.dma_start(m1wsb, m1w_in)
            m2psb = const.tile([120, 128], f32)
            nc.sync.dma_start(m2psb, m2p_in)

            # persistent activations
            qksb = big.tile([128, 2 * KC, N], bf16)   # ct 0..5 = q, 6..11 = k
            vsb = big.tile([128, 8, C], bf16)         # [m%128, m//128, cout]
            attnT = big.tile([128, KC, N], bf16)      # [cout2%128, cout2//128, n]

            # ---------------- QKV ----------------
            with tc.tile_pool(name="xtp", bufs=1) as xtp, \
                 tc.tile_pool(name="qkvps", bufs=3, space="PSUM") as qkvps, \
                 tc.tile_pool(name="vps", bufs=2, space="PSUM") as vps:
                xtsb = xtp.tile([128, KC, N], bf16)
                nc.sync.dma_start(xtsb, xt.rearrange("(kc p) n -> p kc n", p=128))
                wqsb = xtp.tile([128, KC, 3 * C], bf16)
                nc.sync.dma_start(
                    wqsb, wqkvt.rearrange("(kc p) c -> p kc c", p=128))

                # q, k: out[cout_tile, n]
                for ct in range(12):
                    for nh in range(2):
                        ps = qkvps.tile([128, 512], f32, tag="qkv")
                        for kc in range(KC):
                            nc.tensor.matmul(
                                ps,
                                lhsT=wqsb[:, kc, 128 * ct:128 * ct + 128],
                                rhs=xtsb[:, kc, 512 * nh:512 * nh + 512],
                                start=(kc == 0), stop=(kc == KC - 1),
                            )
                        evac(qksb[:, ct, 512 * nh:512 * nh + 512], ps, 512)

                # v: out[n_tile, cout]
                for nt in range(8):
                    ps = vps.tile([128, 768], f32, tag="vps")
                    for half, (c0, c1) in enumerate([(0, 512), (512, 768)]):
                        for kc in range(KC):
                            nc.tensor.matmul(
                                ps[:, c0:c1],
                                lhsT=xtsb[:, kc, 128 * nt:128 * nt + 128],
                                rhs=wqsb[:, kc, 2 * C + c0:2 * C + c1],
                                start=(kc == 0), stop=(kc == KC - 1),
                            )
                    evac(vsb[:, nt, :], ps, 768)

            # ------- attention: software-pipelined over blocks -------
            blocks = _block_layout()
            nblk = len(blocks)

            with tc.tile_pool(name="sallp", bufs=1) as sallp, \
                 tc.tile_pool(name="sintp", bufs=2) as sintp, \
                 tc.tile_pool(name="pintp", bufs=4) as pintp, \
                 tc.tile_pool(name="a2tp", bufs=2) as a2tp, \
                 tc.tile_pool(name="smp", bufs=8) as smp, \
                 tc.tile_pool(name="m2wp", bufs=6) as m2wp, \
                 tc.tile_pool(name="psmix", bufs=2, space="PSUM") as psmix, \
                 tc.tile_pool(name="psA2", bufs=1, space="PSUM") as psA2, \
                 tc.tile_pool(name="psAV", bufs=2, space="PSUM") as psAV:

                sint_t = {}
                a2t_t = {}

                def s_thunks(bi):
                    """Scores of block bi: 12 per-head thunks (2 MMs into a
                    2-bank psum tile + one evac), then the interleave DMAs."""
                    n0, nb, chunks = blocks[bi]
                    sall = sallp.tile([128, 12, N], bf16, tag="sall",
                                      name="sall")
                    out = []

                    def mk(h):
                        def emit():
                            base = 64 * (h % 2)
                            ps = psmix.tile([128, 1024], f32, tag="ps",
                                            name="ps")
                            for mh in range(2):
                                nc.tensor.matmul(
                                    ps[0:nb, 512 * mh:512 * mh + 512],
                                    lhsT=qksb[base:base + 64, h // 2,
                                              n0:n0 + nb],
                                    rhs=qksb[base:base + 64, 6 + h // 2,
                                             512 * mh:512 * mh + 512],
                                    start=True, stop=True,
                                )
                            evac(sall[0:nb, h, :], ps[0:nb, :], 1024)
                        return emit

                    for h in range(12):
                        out.append(mk(h))

                    def interleave():
                        # sint[rr*12+h, g, m] = S_all[g*gs+rr, h, m]
                        sint = sintp.tile([128, 12, N], bf16, tag="sint",
                                          name="sint")
                        sint_t[bi] = sint
                        for (rs, g0, gc, gs) in chunks:
                            for gi in range(gc):
                                nc.gpsimd.dma_start(
                                    out=sint[0:12 * gs, g0 + gi, :],
                                    in_=sall[rs + gi * gs:
                                             rs + (gi + 1) * gs, :, :],
                                )
                    out.append(interleave)
                    return out

                def stage_M_gen(bi):
                    """Mixes of block bi at (group, phase) steps; mix2 (in
                    transposed form, straight into a2t layout) lags mix1 by
                    SKEW steps to hide the ACT exp latency."""
                    n0, nb, chunks = blocks[bi]
                    sint = sint_t.pop(bi)
                    a2t = a2tp.tile([128, 12, 8, 128], bf16, tag="a2t",
                                    name="a2t")
                    a2t_t[bi] = a2t
                    steps = []
                    for (rs, g0, gc, gs) in chunks:
                        for g in range(g0, g0 + gc):
                            steps.append((g, gs))
                    st = {}

                    def mix1(g, gs):
                        rows = 12 * gs
                        pg = pintp.tile([128, N], bf16, tag="pint", name="pg")
                        sm = smp.tile([128, 2], f32, tag="sm", name="sm")
                        st[g] = {"pg": pg, "sm": sm, "w2": None}
                        a1 = psmix.tile([128, 1024], f32, tag="ps", name="a1")
                        for mh in range(2):
                            nc.tensor.matmul(
                                a1[0:rows, 512 * mh:512 * mh + 512],
                                lhsT=m1wsb[0:rows, 0:rows],
                                rhs=sint[0:rows, g, 512 * mh:512 * mh + 512],
                                start=True, stop=True,
                            )
                        _load["act"] += 1050
                        nc.scalar.activation(
                            pg[0:rows, :], a1[0:rows, :], Exp,
                            accum_out=sm[0:rows, 0:1],
                        )
                        _load["dve"] += 150
                        nc.vector.reciprocal(sm[0:rows, 1:2], sm[0:rows, 0:1])
                        w2 = m2wp.tile([128, 128], bf16, tag="m2w", name="w2")
                        st[g]["w2"] = w2
                        nc.gpsimd.tensor_scalar_mul(
                            w2[0:rows, :], m2psb[0:rows, :], sm[0:rows, 1:2])

                    def mix2T(g, gs):
                        # A2^T chunk: out[m', (rr,o)] = sum_rows
                        #   pg[row, m'] * w2[row, (rr,o)] -- m on partitions,
                        # written directly into the a2t (transposed) layout.
                        rows = 12 * gs
                        s = st.pop(g)
                        a2 = psA2.tile([128, 8, 128], f32, tag="psA2",
                                       name="a2")
                        for c in range(8):
                            nc.tensor.matmul(
                                a2[:, c, :],
                                lhsT=s["pg"][0:rows, 128 * c:128 * c + 128],
                                rhs=s["w2"][0:rows, :],
                                start=True, stop=True,
                            )
                            if c == 3:
                                evac(a2t[:, g, 0:4, :], a2[:, 0:4, :], 512)
                        evac(a2t[:, g, 4:8, :], a2[:, 4:8, :], 512)

                    SKEW = 2
                    for i in range(len(steps) + SKEW):
                        if i < len(steps):
                            mix1(*steps[i])
                        if i >= SKEW:
                            mix2T(*steps[i - SKEW])
                        yield

                def av_thunks(bi):
                    """attention@V of block bi: 6 column-packed head-pair
                    chains accumulating into shared 1-bank psum tiles."""
                    n0, nb, chunks = blocks[bi]
                    a2t = a2t_t.pop(bi)
                    out = []
                    tiles = {}

                    def mk(j):
                        def emit():
                            grp, jj = divmod(j, 4)
                            npair = 4 if grp == 0 else 2
                            if jj == 0:
                                tiles[grp] = psAV.tile(
                                    [128, npair, 128], f32, tag="psAV",
                                    name="av", padded_shape=[128, 4, 128])
                            av = tiles[grp]
                            for half in range(2):
                                o = 2 * j + half
                                for (rs, g0, gc, gs) in chunks:
                                    for c in range(8):
                                        nc.tensor.matmul(
                                            av[64 * half:64 * half + 64, jj,
                                               rs:rs + gc * gs],
                                            lhsT=vsb[:, c, 64 * o:64 * o + 64],
                                            rhs=a2t[:, g0:g0 + gc, c,
                                                    o:o + 12 * (gs - 1) + 1:12],
                                            start=(c == 0), stop=(c == 7),
                                        )
                            if jj == npair - 1:
                                evac(attnT[:, 4 * grp:4 * grp + npair,
                                           n0:n0 + nb],
                                     av[:, :, 0:nb], npair * nb)
                        return emit

                    for j in range(6):
                        out.append(mk(j))
                    return out

                for it in range(nblk + 2):
                    gen = stage_M_gen(it - 1) if 1 <= it <= nblk else None
                    avs = av_thunks(it - 2) if 2 <= it <= nblk + 1 else []
                    ss = s_thunks(it) if it < nblk else []
                    if gen is None:
                        for t in ss:
                            t()
                        for t in avs:
                            t()
                    else:
                        k, ai, si = 0, 0, 0
                        for _ in gen:
                            k += 1
                            if si < len(ss):
                                ss[si]()
                                si += 1
                            if ai < len(avs) and k % 2 == 0:
                                avs[ai]()
                                ai += 1
                        while si < len(ss):
                            ss[si]()
                            si += 1
                        while ai < len(avs):
                            avs[ai]()
                            ai += 1

            # ---------------- proj ----------------
            with tc.tile_pool(name="projps", bufs=3, space="PSUM") as pjp, \
                 tc.tile_pool(name="outp", bufs=3) as outp:
                wpsb = outp.tile([128, KC, C], bf16)
                nc.sync.dma_start(
                    wpsb, wprojt.rearrange("(kc p) c -> p kc c", p=128))
                od = out_d.rearrange("(ct p) n -> p ct n", p=128)
                for ct in range(KC):
                    for nh in range(2):
                        ps = pjp.tile([128, 512], f32, tag="pj")
                        for kc in range(KC):
                            nc.tensor.matmul(
                                ps,
                                lhsT=wpsb[:, kc, 128 * ct:128 * ct + 128],
                                rhs=attnT[:, kc, 512 * nh:512 * nh + 512],
                                start=(kc == 0), stop=(kc == KC - 1),
                            )
                        ob = outp.tile([128, 512], f32, tag="ob")
                        evac(ob, ps, 512)
                        nc.sync.dma_start(
                            od[:, ct, 512 * nh:512 * nh + 512], ob)

    nc.compile()
    return nc


def _mix_weights(conv_l_w, conv_w_w):
    """Host-built mix lhsT matrices, row map r = rr*12 + h.

    m1w[rr*12+h, rr*12+o] = SCALE * conv_l[o, h]   (lhsT for mix1)
    m2p[rr*12+h, rr*12+o] = conv_w[o, h]           (pattern, f32; scaled
        per-group on device by 1/softmax_sum per row; 128 cols, cols >= 120
        are zero so A2 psum rows [rows:128) are zeros)
    The gs=4 ragged group uses the leading [48, 48] / [48, :] slices.
    """
    m1 = np.zeros((120, 120), np.float32)
    m2 = np.zeros((120, 128), np.float32)
    for rr in range(G):
        for h in range(12):
            for o in range(12):
                m1[rr * 12 + h, rr * 12 + o] = SCALE * conv_l_w[o, h]
                m2[rr * 12 + h, rr * 12 + o] = conv_w_w[o, h]
    return m1.astype(BF16), m2.astype(np.float32)


def _run(x, w_qkv, w_proj, b_proj, conv_l_w, conv_w_w, **spmd_kwargs):
    global _cached
    from concourse import bass_utils

    x = np.asarray(x, np.float32)
    w_qkv = np.asarray(w_qkv, np.float32)
    w_proj = np.asarray(w_proj, np.float32)
    b_proj = np.asarray(b_proj, np.float32)
    conv_l_w = np.asarray(conv_l_w, np.float32)
    conv_w_w = np.asarray(conv_w_w, np.float32)

    if _cached is None:
        _cached = _build_program()
    nc = _cached

    m1w, m2p = _mix_weights(conv_l_w, conv_w_w)
    wqkvt = np.ascontiguousarray(w_qkv.T).astype(BF16)
    wprojt = np.ascontiguousarray(w_proj.T).astype(BF16)

    in_maps = []
    for b in range(B):
        in_maps.append({
            "xt": np.ascontiguousarray(x[b].T).astype(BF16),
            "wqkvt": wqkvt,
            "wprojt": wprojt,
            "m1w": m1w,
            "m2p": m2p,
        })

    res = bass_utils.run_bass_kernel_spmd(
        nc, in_maps, core_ids=list(range(B)), **spmd_kwargs)
    out = np.stack([res.results[b]["out"].T for b in range(B)])  # [B, N, C]
    return (out + b_proj[None, None, :]).astype(np.float32), res


def kernel(x, w_qkv, w_proj, b_proj, conv_l_w, conv_w_w):
    out, _ = _run(x, w_qkv, w_proj, b_proj, conv_l_w, conv_w_w)
    return out


# revision 13
# speedup vs baseline: 1.2251x; 1.0563x over previous
"""MiniAttention Trainium2 Bass kernel.

Problem: B=8, N=1024, C=768, H=12, D=64.
  qkv = x @ w_qkv.T ; q,k,v heads ; S = (q*SCALE) @ k.T per head
  A1 = conv_l-mix over heads ; P = softmax_m(A1) ; A2 = conv_w-mix over heads
  out = (A2 @ v per head) @ w_proj.T + b_proj
Sharding: pure batch-parallel, 1 batch element per NeuronCore (8 cores).

Per-core design (PE matmuls in bf16, f32 accumulation):
  - Host passes x^T, w_qkv^T, w_proj^T (transposed on host, bf16).
  - Scores per head h evac'd into S_all [block_row, (h, m)]; head-interleave
    into groups of G=10 queries with row map r = rr*12 + h via ONE SWDGE
    DMA per group (engines execute in order, and HWDGE dma_start costs
    ~600ns of Sync occupancy each, so both queue choice and instruction
    count matter).
  - conv_l (SCALE folded) as constant rr-block-diagonal lhsT; exp on ACT
    with accum_out giving softmax sums; normalization folded into the
    per-group conv_w lhsT (rows scaled by 1/sum, built on GpSimd).
  - A2 -> xbar DMA-transpose -> attention@V contracts m at K=128 with
    column-packed head pairs (full 128-partition PSUM tile).
  - All engines execute their instruction streams IN ORDER, so the block
    loop is software-pipelined by emission order: per iteration emit
    mixes(b-1) with a 3-half skew (mix2 lags mix1 so ACT exp latency is
    hidden) and the AV chains of block b-2 injected between mix steps,
    then scores(b) last. PSUM: scores+mix1 share one 4-buffer ring
    (tag "ps"), mix2 2 banks, AV 2 banks = 8 banks total.
  - PSUM->SBUF evacs are greedily balanced between ACT and DVE by
    accumulated estimated cost (exp is pinned to ACT, small ops to GpSimd).
"""

import numpy as np
import ml_dtypes

B, N, C, H = 8, 1024, 768, 12
D = C // H
SCALE = D ** -0.5
G = 10          # queries per mix group
NB = 120        # queries per block (12 groups)
NBLK = 8        # full blocks; last block is ragged: 6 groups of 10 + 1 of 4
BF16 = ml_dtypes.bfloat16

_cached = None


def _block_layout():
    """Returns list of blocks: (n0, nb, chunks) where chunks is a list of
    (row_start, g_start, g_count, g_size) describing the query groups."""
    blocks = []
    for b in range(NBLK):
        blocks.append((b * NB, NB, [(0, 0, 12, G)]))
    # ragged tail: n in [960, 1024) = 6 groups of 10 + 1 group of 4
    blocks.append((960, 64, [(0, 0, 6, G), (60, 6, 1, 4)]))
    return blocks


def _build_program():
    import concourse.tile as tile
    from concourse import bacc, mybir

    f32 = mybir.dt.float32
    bf16 = mybir.dt.bfloat16
    Exp = mybir.ActivationFunctionType.Exp

    nc = bacc.Bacc("TRN2", target_bir_lowering=False, debug=False)

    xt = nc.dram_tensor("xt", [C, N], bf16, kind="ExternalInput").ap()
    wqkvt = nc.dram_tensor("wqkvt", [C, 3 * C], bf16, kind="ExternalInput").ap()
    wprojt = nc.dram_tensor("wprojt", [C, C], bf16, kind="ExternalInput").ap()
    m1w_in = nc.dram_tensor("m1w", [12 * G, 128], bf16, kind="ExternalInput").ap()
    m2p_in = nc.dram_tensor("m2p", [128, 128], f32, kind="ExternalInput").ap()
    out_d = nc.dram_tensor("out", [C, N], f32, kind="ExternalOutput").ap()

    KC = C // 128  # 6 contraction chunks

    # greedy ACT/DVE balance by estimated occupancy (ns)
    _load = {"act": 0.0, "dve": 0.0}

    def evac(dst, src, n):
        """PSUM->SBUF copy of [rows, n]; pick the less-loaded engine."""
        if _load["act"] + 200 + n / 1.2 <= _load["dve"] + 200 + n / 0.96:
            _load["act"] += 200 + n / 1.2
            nc.scalar.copy(dst, src)
        else:
            _load["dve"] += 200 + n / 0.96
            nc.vector.tensor_copy(dst, src)

    with tile.TileContext(nc) as tc:
        with tc.tile_pool(name="const", bufs=1) as const, \
             tc.tile_pool(name="big", bufs=1) as big:

            m1wsb = const.tile([120, 128], bf16)
            nc.sync.dma_start(m1wsb, m1w_in)
            m2psb = const.tile([128, 128], f32)
            nc.sync.dma_start(m2psb, m2p_in)

            # persistent activations
            qksb = big.tile([128, 2 * KC, N], bf16)   # ct 0..5 = q, 6..11 = k
            vsb = big.tile([128, 8, C], bf16)         # [m%128, m//128, cout]
            attnT = big.tile([128, KC, N], bf16)      # [cout2%128, cout2//128, n]

            # ---------------- QKV ----------------
            with tc.tile_pool(name="xtp", bufs=1) as xtp, \
                 tc.tile_pool(name="qkvps", bufs=3, space="PSUM") as qkvps, \
                 tc.tile_pool(name="vps", bufs=2, space="PSUM") as vps:
                xtsb = xtp.tile([128, KC, N], bf16)
                nc.sync.dma_start(xtsb, xt.rearrange("(kc p) n -> p kc n", p=128))
                wqsb = xtp.tile([128, KC, 3 * C], bf16)
                nc.sync.dma_start(
                    wqsb, wqkvt.rearrange("(kc p) c -> p kc c", p=128))

                # q, k: out[cout_tile, n]
                for ct in range(12):
                    for nh in range(2):
                        ps = qkvps.tile([128, 512], f32, tag="qkv")
                        for kc in range(KC):
                            nc.tensor.matmul(
                                ps,
                                lhsT=wqsb[:, kc, 128 * ct:128 * ct + 128],
                                rhs=xtsb[:, kc, 512 * nh:512 * nh + 512],
                                start=(kc == 0), stop=(kc == KC - 1),
                            )
                        evac(qksb[:, ct, 512 * nh:512 * nh + 512], ps, 512)

                # v: out[n_tile, cout]
                for nt in range(8):
                    ps = vps.tile([128, 768], f32, tag="vps")
                    for half, (c0, c1) in enumerate([(0, 512), (512, 768)]):
                        for kc in range(KC):
                            nc.tensor.matmul(
                                ps[:, c0:c1],
                                lhsT=xtsb[:, kc, 128 * nt:128 * nt + 128],
                                rhs=wqsb[:, kc, 2 * C + c0:2 * C + c1],
                                start=(kc == 0), stop=(kc == KC - 1),
                            )
                    evac(vsb[:, nt, :], ps, 768)

            # ------- attention: software-pipelined over blocks -------
            # Per iteration (emission order == execution order per engine):
            #   1. dense PE burst: AV chains of block b-2 woven with score
            #      matmul pairs of block b (keeps the PE busy enough for the
            #      HAM clock-gate to hold 2.4 GHz; score evacs drain on
            #      ACT/DVE underneath),
            #   2. interleave DMAs of block b (sync/gpsimd alternating),
            #   3. mix steps of block b-1 (mix2T lags mix1 by SKEW to hide
            #      the ACT exp latency).
            # Mix matmuls carry 128 weight columns so the compiler enables
            # FWL (4x faster LDWEIGHTS): m1w is column-padded with zeros, so
            # a1 rows [rows:128) are written as zeros, exp turns them into
            # finite 1s, and m2p's zero rows [120:128) zero them out of A2.
            blocks = _block_layout()
            nblk = len(blocks)

            with tc.tile_pool(name="sallp", bufs=1) as sallp, \
                 tc.tile_pool(name="sintp", bufs=2) as sintp, \
                 tc.tile_pool(name="pintp", bufs=4) as pintp, \
                 tc.tile_pool(name="a2tp", bufs=2) as a2tp, \
                 tc.tile_pool(name="smp", bufs=8) as smp, \
                 tc.tile_pool(name="m2wp", bufs=6) as m2wp, \
                 tc.tile_pool(name="psmix", bufs=2, space="PSUM") as psmix, \
                 tc.tile_pool(name="psA2", bufs=1, space="PSUM") as psA2, \
                 tc.tile_pool(name="psAV", bufs=2, space="PSUM") as psAV:

                sint_t = {}
                a2t_t = {}
                _dq = [0]

                def s_thunks(bi):
                    """Scores of block bi: 12 per-head thunks (2 MMs into a
                    2-bank psum tile + one evac), then the interleave DMAs."""
                    n0, nb, chunks = blocks[bi]
                    sall = sallp.tile([128, 12, N], bf16, tag="sall",
                                      name="sall")
                    out = []

                    def mk(h):
                        def emit():
                            base = 64 * (h % 2)
                            ps = psmix.tile([128, 1024], f32, tag="ps",
                                            name="ps")
                            for mh in range(2):
                                nc.tensor.matmul(
                                    ps[0:nb, 512 * mh:512 * mh + 512],
                                    lhsT=qksb[base:base + 64, h // 2,
                                              n0:n0 + nb],
                                    rhs=qksb[base:base + 64, 6 + h // 2,
                                             512 * mh:512 * mh + 512],
                                    start=True, stop=True,
                                )
                            evac(sall[0:nb, h, :], ps[0:nb, :], 1024)
                        return emit

                    for h in range(12):
                        out.append(mk(h))

                    def interleave():
                        # sint[rr*12+h, g, m] = S_all[g*gs+rr, h, m]
                        sint = sintp.tile([128, 12, N], bf16, tag="sint",
                                          name="sint")
                        sint_t[bi] = sint
                        for (rs, g0, gc, gs) in chunks:
                            for gi in range(gc):
                                eng = nc.sync if _dq[0] % 2 == 0 else nc.gpsimd
                                _dq[0] += 1
                                eng.dma_start(
                                    out=sint[0:12 * gs, g0 + gi, :],
                                    in_=sall[rs + gi * gs:
                                             rs + (gi + 1) * gs, :, :],
                                )
                    out.append(interleave)
                    return out

                def stage_M_gen(bi):
                    """Mixes of block bi at group steps; mix2 (transposed,
                    straight into a2t layout) lags mix1 by SKEW steps."""
                    n0, nb, chunks = blocks[bi]
                    sint = sint_t.pop(bi)
                    a2t = a2tp.tile([128, 12, 8, 128], bf16, tag="a2t",
                                    name="a2t")
                    a2t_t[bi] = a2t
                    steps = []
                    for (rs, g0, gc, gs) in chunks:
                        for g in range(g0, g0 + gc):
                            steps.append((g, gs))
                    st = {}

                    def mix1(g, gs):
                        rows = 12 * gs
                        pg = pintp.tile([128, N], bf16, tag="pint", name="pg")
                        sm = smp.tile([128, 2], f32, tag="sm", name="sm")
                        st[g] = {"pg": pg, "sm": sm, "w2": None, "gs": gs}
                        a1 = psmix.tile([128, 1024], f32, tag="ps", name="a1")
                        for mh in range(2):
                            nc.tensor.matmul(
                                a1[:, 512 * mh:512 * mh + 512],
                                lhsT=m1wsb[0:rows, 0:128],
                                rhs=sint[0:rows, g, 512 * mh:512 * mh + 512],
                                start=True, stop=True,
                            )
                        _load["act"] += 1050
                        nc.scalar.activation(
                            pg, a1, Exp,
                            accum_out=sm[:, 0:1],
                        )
                        _load["dve"] += 150
                        nc.vector.reciprocal(sm[:, 1:2], sm[:, 0:1])
                        w2 = m2wp.tile([128, 128], bf16, tag="m2w", name="w2")
                        st[g]["w2"] = w2
                        kr = 128 if gs == G else 12 * gs
                        nc.gpsimd.tensor_scalar_mul(
                            w2[0:kr, :], m2psb[0:kr, :], sm[0:kr, 1:2])

                    def mix2T(g, gs):
                        # A2^T chunk: out[m', (rr,o)] = sum_rows
                        #   pg[row, m'] * w2[row, (rr,o)] -- m on partitions,
                        # written directly into the a2t (transposed) layout.
                        # K=128 (padded rows contribute exp(0)*0) -> FWL.
                        s = st.pop(g)
                        kr = 128 if gs == G else 12 * gs
                        a2 = psA2.tile([128, 8, 128], f32, tag="psA2",
                                       name="a2")
                        for c in range(8):
                            nc.tensor.matmul(
                                a2[:, c, :],
                                lhsT=s["pg"][0:kr, 128 * c:128 * c + 128],
                                rhs=s["w2"][0:kr, :],
                                start=True, stop=True,
                            )
                            if c == 3:
                                evac(a2t[:, g, 0:4, :], a2[:, 0:4, :], 512)
                        evac(a2t[:, g, 4:8, :], a2[:, 4:8, :], 512)

                    SKEW = 2
                    for i in range(len(steps) + SKEW):
                        if i < len(steps):
                            mix1(*steps[i])
                        if i >= SKEW:
                            mix2T(*steps[i - SKEW])
                        yield

                def av_thunks(bi):
                    """attention@V of block bi: 6 column-packed head-pair
                    chains accumulating into shared 1-bank psum tiles."""
                    n0, nb, chunks = blocks[bi]
                    a2t = a2t_t.pop(bi)
                    out = []
                    tiles = {}

                    def mk(j):
                        def emit():
                            grp, jj = divmod(j, 4)
                            npair = 4 if grp == 0 else 2
                            if jj == 0:
                                tiles[grp] = psAV.tile(
                                    [128, npair, 128], f32, tag="psAV",
                                    name="av", padded_shape=[128, 4, 128])
                            av = tiles[grp]
                            for half in range(2):
                                o = 2 * j + half
                                for (rs, g0, gc, gs) in chunks:
                                    for c in range(8):
                                        nc.tensor.matmul(
                                            av[64 * half:64 * half + 64, jj,
                                               rs:rs + gc * gs],
                                            lhsT=vsb[:, c, 64 * o:64 * o + 64],
                                            rhs=a2t[:, g0:g0 + gc, c,
                                                    o:o + 12 * (gs - 1) + 1:12],
                                            start=(c == 0), stop=(c == 7),
                                        )
                            if jj == npair - 1:
                                evac(attnT[:, 4 * grp:4 * grp + npair,
                                           n0:n0 + nb],
                                     av[:, :, 0:nb], npair * nb)
                        return emit

                    for j in range(6):
                        out.append(mk(j))
                    return out

                for it in range(nblk + 2):
                    avs = av_thunks(it - 2) if 2 <= it <= nblk + 1 else []
                    ss = s_thunks(it) if it < nblk else []
                    # 1. dense PE burst: AV chains woven with score pairs
                    si = 0
                    for t in avs:
                        t()
                        for _ in range(2):
                            if si < len(ss) - 1:
                                ss[si]()
                                si += 1
                    while si < len(ss) - 1:
                        ss[si]()
                        si += 1
                    # 2. interleave DMAs of block `it`
                    if ss:
                        ss[-1]()
                    # 3. mixes of block it-1
                    if 1 <= it <= nblk:
                        for _ in stage_M_gen(it - 1):
                            pass

            # ---------------- proj ----------------
            with tc.tile_pool(name="projps", bufs=3, space="PSUM") as pjp, \
                 tc.tile_pool(name="outp", bufs=3) as outp:
                wpsb = outp.tile([128, KC, C], bf16)
                nc.sync.dma_start(
                    wpsb, wprojt.rearrange("(kc p) c -> p kc c", p=128))
                od = out_d.rearrange("(ct p) n -> p ct n", p=128)
                for ct in range(KC):
                    for nh in range(2):
                        ps = pjp.tile([128, 512], f32, tag="pj")
                        for kc in range(KC):
                            nc.tensor.matmul(
                                ps,
                                lhsT=wpsb[:, kc, 128 * ct:128 * ct + 128],
                                rhs=attnT[:, kc, 512 * nh:512 * nh + 512],
                                start=(kc == 0), stop=(kc == KC - 1),
                            )
                        ob = outp.tile([128, 512], f32, tag="ob")
                        evac(ob, ps, 512)
                        nc.sync.dma_start(
                            od[:, ct, 512 * nh:512 * nh + 512], ob)

    nc.compile()
    return nc


def _mix_weights(conv_l_w, conv_w_w):
    """Host-built mix lhsT matrices, row map r = rr*12 + h.

    m1w[rr*12+h, rr*12+o] = SCALE * conv_l[o, h]   (lhsT for mix1)
    m2p[rr*12+h, rr*12+o] = conv_w[o, h]           (pattern, f32; scaled
        per-group on device by 1/softmax_sum per row; 128 cols, cols >= 120
        are zero so A2 psum rows [rows:128) are zeros)
    The gs=4 ragged group uses the leading [48, 48] / [48, :] slices.
    """
    m1 = np.zeros((120, 128), np.float32)
    m2 = np.zeros((128, 128), np.float32)
    for rr in range(G):
        for h in range(12):
            for o in range(12):
                m1[rr * 12 + h, rr * 12 + o] = SCALE * conv_l_w[o, h]
                m2[rr * 12 + h, rr * 12 + o] = conv_w_w[o, h]
    return m1.astype(BF16), m2.astype(np.float32)


def _run(x, w_qkv, w_proj, b_proj, conv_l_w, conv_w_w, **spmd_kwargs):
    global _cached
    from concourse import bass_utils

    x = np.asarray(x, np.float32)
    w_qkv = np.asarray(w_qkv, np.float32)
    w_proj = np.asarray(w_proj, np.float32)
    b_proj = np.asarray(b_proj, np.float32)
    conv_l_w = np.asarray(conv_l_w, np.float32)
    conv_w_w = np.asarray(conv_w_w, np.float32)

    if _cached is None:
        _cached = _build_program()
    nc = _cached

    m1w, m2p = _mix_weights(conv_l_w, conv_w_w)
    wqkvt = np.ascontiguousarray(w_qkv.T).astype(BF16)
    wprojt = np.ascontiguousarray(w_proj.T).astype(BF16)

    in_maps = []
    for b in range(B):
        in_maps.append({
            "xt": np.ascontiguousarray(x[b].T).astype(BF16),
            "wqkvt": wqkvt,
            "wprojt": wprojt,
            "m1w": m1w,
            "m2p": m2p,
        })

    res = bass_utils.run_bass_kernel_spmd(
        nc, in_maps, core_ids=list(range(B)), **spmd_kwargs)
    out = np.stack([res.results[b]["out"].T for b in range(B)])  # [B, N, C]
    return (out + b_proj[None, None, :]).astype(np.float32), res


def kernel(x, w_qkv, w_proj, b_proj, conv_l_w, conv_w_w):
    out, _ = _run(x, w_qkv, w_proj, b_proj, conv_l_w, conv_w_w)
    return out
